# revision 5
# baseline (speedup 1.0000x reference)
"""CoSTCo model kernel for 8x Trainium2 NeuronCores.

Math: out[b] = relu(wfc2 @ relu(wfc1 @ h2[b] + bfc1) + bfc2), where
  h2[b] = relu(Q02[i0[b]*64 + i2[b]] + Q1[i1[b]])
  Q_m   = relu(emb_m @ w1.T + b1) @ w2[:, :, m].T        (weight folding)
  Q02[i*64+j] = Q0[i] + Q2[j] + b2                       (pair fusion)

conv1 (over rank) and conv2 (over modes) act linearly on each gathered
embedding row, so they fold into per-table lookup matrices Q_m computed
once on the host (tables are tiny: 339/5825/64 rows). Modes 0 and 2 fuse
further into one 21696-row pair table, so the device does 2 dma_gathers
per batch element instead of 3.

Device per 512-batch block: 1 DVE add, 8 PE transposes into [channel,
batch] layout, relu(+bias) drains, and the MLP (256->256->1) on the
tensor engine.

Startup path: one packed idx DMA (p-major contiguous layout), two packed
const DMAs, so the first gather issues within a few microseconds. The
gather chunk is 2048 indices to amortize the ~1us fixed SWDGE
descriptor-generation cost per gather instruction.

Sharding: pure data parallel over the batch dim, 16384 elements per core.
"""

import sys
import types

sys.path.insert(0, "/opt/trn_rl_repo")

import ml_dtypes
import numpy as np

# ---------------------------------------------------------------- constants
B = 131072
N_CORES = 8
BPC = B // N_CORES          # 16384 batch elements per core
CHUNK = 1024                # idx per dma_gather instruction (Q7 scratch caps this)
RANK = 128
C = 256                     # channels
FIELD_DIMS = (339, 5825, 64)
F02 = FIELD_DIMS[0] * FIELD_DIMS[2]   # fused pair-table rows
NSWQ = 4                    # SWDGE queues in use
SCRATCH = 65536             # dynamic DMA scratch bytes per partition

TDT = "bf16"                # gather-table dtype
MDT = "bf16"                # matmul/activation dtype


def _install_ntff_hook():
    """antenv in this image lacks axon_hooks; inject it and register the
    ctypes NTFF profiling hook so trace=True works under axon."""
    import antenv

    if "antenv.axon_hooks" in sys.modules:
        return
    mod = types.ModuleType("antenv.axon_hooks")
    mod._hook = None
    mod.set_axon_ntff_profile_hook = lambda h: setattr(mod, "_hook", h)
    mod.get_axon_ntff_profile_hook = lambda: mod._hook
    sys.modules["antenv.axon_hooks"] = mod
    antenv.axon_hooks = mod
    try:
        from trn_agent_boot.trn_boot import _ntff_profile_via_ctypes

        mod._hook = _ntff_profile_via_ctypes("/opt/axon/libaxon_pjrt.so")
    except Exception:
        pass


_NC_CACHE = {}


def _build(bpc=BPC, chunk=CHUNK):
    """Build + compile the per-core Bass program. Identical on all cores;
    per-core data arrives via in_maps."""
    import concourse.tile as tile
    from concourse import bacc, mybir

    key = (bpc, chunk)
    if key in _NC_CACHE:
        return _NC_CACHE[key]

    f32 = mybir.dt.float32
    bf16 = mybir.dt.bfloat16
    i16 = mybir.dt.int16
    Alu = mybir.AluOpType
    Act = mybir.ActivationFunctionType
    nchunk = bpc // chunk
    nblk = chunk // 512
    ngrp = chunk // 128
    cw = chunk // 16            # idx columns per chunk (per table)
    iw = bpc // 16              # idx columns per table

    nc = bacc.Bacc("TRN2", target_bir_lowering=False, debug=False,
                   num_devices=N_CORES, num_swdge_queues=NSWQ,
                   dynamic_dma_scratch_size=SCRATCH)

    # DRAM inputs (per-core shards / replicated folded weights)
    q02_dram = nc.dram_tensor("q02", [F02, C], bf16, kind="ExternalInput")
    q1_dram = nc.dram_tensor("q1", [FIELD_DIMS[1], C], bf16,
                             kind="ExternalInput")
    # both tables' wrapped idx packed p-major: [128, 2*iw] (table-major cols)
    idx_dram = nc.dram_tensor("idxw", [128, 2 * iw], i16,
                              kind="ExternalInput")
    # packed consts: [128, 128 ident | 512 w1t | 2 w2t] bf16, [128, 3] f32
    cwb_dram = nc.dram_tensor("cwb", [128, 642], bf16, kind="ExternalInput")
    cbf_dram = nc.dram_tensor("cbf", [128, 3], f32, kind="ExternalInput")
    out_dram = nc.dram_tensor("out", [bpc], f32, kind="ExternalOutput")
    out_view = out_dram.ap().rearrange("(c n) -> c n", n=chunk)

    with tile.TileContext(nc) as tc:
        with (
            tc.tile_pool(name="const", bufs=1) as const_pool,
            tc.tile_pool(name="gat", bufs=4) as gat_pool,
            tc.tile_pool(name="sum", bufs=3) as sum_pool,
            tc.tile_pool(name="act", bufs=3) as act_pool,
            tc.tile_pool(name="stage", bufs=2) as stage_pool,
            tc.tile_pool(name="pt", bufs=4, space="PSUM") as pt_pool,
            tc.tile_pool(name="ph", bufs=3, space="PSUM") as ph_pool,
            tc.tile_pool(name="po", bufs=1, space="PSUM") as po_pool,
        ):
            # --- idx first (gates the first gather), then consts.
            # chunk-0 idx slices load separately so the first gathers fire
            # without waiting for the full idx tile.
            idxs = const_pool.tile([128, 2 * iw], i16)
            for m in range(2):
                nc.sync.dma_start(idxs[:, m * iw:m * iw + cw],
                                  idx_dram.ap()[:, m * iw:m * iw + cw])
            for m in range(2):
                nc.sync.dma_start(idxs[:, m * iw + cw:(m + 1) * iw],
                                  idx_dram.ap()[:, m * iw + cw:(m + 1) * iw])
            cwb = const_pool.tile([128, 642], bf16)
            nc.sync.dma_start(cwb[:], cwb_dram.ap())
            cbf = const_pool.tile([128, 3], f32)
            nc.sync.dma_start(cbf[:], cbf_dram.ap())

            ident = cwb[:, 0:128]
            w1t = [cwb[:, 128 + j * C:128 + (j + 1) * C] for j in range(2)]
            w2t = cwb[:, 640:642]
            b1s = cbf[:, 0:2]
            b3s = cbf[0:1, 2:3]

            for ch in range(nchunk):
                # --- gather table rows for this chunk: [128, ngrp, 256]
                g = []
                for m, src in enumerate((q02_dram, q1_dram)):
                    dst = gat_pool.tile([128, ngrp, C], bf16, tag=f"g{m}")
                    nc.gpsimd.dma_gather(
                        dst[:], src.ap(),
                        idxs[:, m * iw + ch * cw:m * iw + (ch + 1) * cw],
                        chunk, chunk, C,
                        queue_num=(2 * ch + m) % NSWQ,
                    )
                    g.append(dst)

                stage = stage_pool.tile([1, chunk], f32)
                for blk in range(nblk):
                    gs = slice(4 * blk, 4 * blk + 4)
                    # --- s = g02 + g1  (row layout [batch, channel])
                    s = sum_pool.tile([128, 4, C], bf16)
                    nc.vector.tensor_tensor(s[:], g[0][:, gs, :],
                                            g[1][:, gs, :], Alu.add)
                    # --- transpose to [channel, batch], 2 halves of 128
                    h2 = []
                    for h in range(2):
                        ps = pt_pool.tile([128, 512], bf16, tag="pt")
                        for grp in range(4):
                            nc.tensor.transpose(
                                ps[:, grp * 128:(grp + 1) * 128],
                                s[:, grp, h * 128:(h + 1) * 128],
                                ident,
                            )
                        # --- h2 = relu(sum)  (b2 folded into q02 on host)
                        hs = act_pool.tile([128, 512], bf16, tag=f"h2{h}")
                        nc.scalar.activation(hs[:], ps[:], Act.Relu)
                        h2.append(hs[:])
                    # --- fc1: h3 = relu(wfc1 @ h2 + bfc1), 2 output halves
                    h3 = []
                    for h in range(2):
                        ph = ph_pool.tile([128, 512], f32, tag="ph")
                        for j in range(2):
                            nc.tensor.matmul(
                                ph[:],
                                w1t[j][:, h * 128:(h + 1) * 128],
                                h2[j],
                                start=(j == 0), stop=(j == 1),
                            )
                        hs = act_pool.tile([128, 512], bf16, tag=f"h3{h}")
                        nc.vector.tensor_scalar(hs[:], ph[:],
                                                b1s[:, h:h + 1], 0.0,
                                                Alu.add, Alu.max)
                        h3.append(hs)
                    # --- fc2: out = relu(wfc2 @ h3 + bfc2)
                    po = po_pool.tile([128, 512], f32, tag="po")
                    for j in range(2):
                        nc.tensor.matmul(po[0:1, :],
                                         w2t[:, j:j + 1],
                                         h3[j][:],
                                         start=(j == 0), stop=(j == 1))
                    nc.scalar.activation(
                        stage[0:1, blk * 512:(blk + 1) * 512], po[0:1, :],
                        Act.Relu, bias=b3s)
                nc.sync.dma_start(out_view[ch:ch + 1, :], stage[:])

    nc.compile()
    _NC_CACHE[key] = nc
    return nc


def _fold_tables(inputs):
    """Q_m = relu(emb_m @ w1.T + b1) @ w2[:,:,m].T in float64, then the
    mode-0/2 pair fusion Q02[i*64+j] = Q0[i] + Q2[j] + b2."""
    w1_ = np.asarray(inputs["w1"]).astype(np.float64)
    b1_ = np.asarray(inputs["b1"]).astype(np.float64)
    w2 = np.asarray(inputs["w2"])
    qs = []
    for m, emb in enumerate((inputs["emb0"], inputs["emb1"], inputs["emb2"])):
        r = np.maximum(np.asarray(emb).astype(np.float64) @ w1_.T + b1_, 0.0)
        qs.append(r @ w2[:, :, m].astype(np.float64).T)
    q02 = (qs[0][:, None, :] + qs[2][None, :, :]
           + np.asarray(inputs["b2"]).astype(np.float64)).reshape(F02, C)
    return q02, qs[1]


def _make_common(inputs):
    bf = ml_dtypes.bfloat16
    q02, q1 = _fold_tables(inputs)
    ident = np.eye(128, dtype=bf)
    w1t = np.asarray(inputs["wfc1"]).T.astype(bf).reshape(2, 128, C)
    # cwb free layout: [ident 128 | w1t[j=0] 256 | w1t[j=1] 256 | w2t 2]
    cwb = np.concatenate(
        [ident, w1t[0], w1t[1],
         np.asarray(inputs["wfc2"]).reshape(C).astype(bf).reshape(2, 128).T],
        axis=1)
    cbf = np.zeros((128, 3), np.float32)
    cbf[:, 0:2] = np.asarray(inputs["bfc1"]).astype(np.float32).reshape(2, 128).T
    cbf[0, 2] = float(np.asarray(inputs["bfc2"]).reshape(()))
    return {
        "q02": np.ascontiguousarray(q02.astype(bf)),
        "q1": np.ascontiguousarray(q1.astype(bf)),
        "cwb": np.ascontiguousarray(cwb),
        "cbf": cbf,
    }


def _wrap_idx(idx, chunk):
    """Wrap a 1-D int array into dma_gather's [128, n/16] int16 layout,
    chunk by chunk: logical position k of chunk c lives at
    [k % 16, c*chunk/16 + k // 16], replicated across the 8 Q7 cores."""
    n = idx.shape[0]
    w = (idx.reshape(n // chunk, chunk // 16, 16)
         .transpose(0, 2, 1).reshape(n // chunk, 16, chunk // 16))
    wrapped = np.concatenate(list(w), axis=1).astype(np.int16)  # [16, n/16]
    return np.tile(wrapped, (8, 1))                             # [128, n/16]


def _make_idxw(shard, chunk=CHUNK):
    """shard: [n, 3] int indices -> ([128, 2*n/16] int16 packed layout,
    order): cols [0, n/16) are the fused mode-0/2 index, cols [n/16, 2n/16)
    the mode-1 index. The batch is sorted by the fused index so the
    big-table HBM reads are sequential-ish; `order` maps device position ->
    original row (undo with out[order] = device_out)."""
    i02 = np.asarray(shard[:, 0]).astype(np.int64) * FIELD_DIMS[2] \
        + np.asarray(shard[:, 2])
    i1 = np.asarray(shard[:, 1]).astype(np.int64)
    order = np.arange(i02.shape[0])
    return np.concatenate([_wrap_idx(i02[order], chunk),
                           _wrap_idx(i1[order], chunk)], axis=1), order


def _run(inputs, trace=False, trace_kwargs=None):
    _install_ntff_hook()
    from concourse.bass_utils import run_bass_kernel_spmd

    nc = _build()
    common = _make_common(inputs)
    indices = np.asarray(inputs["indices"])
    in_maps, orders = [], []
    for c in range(N_CORES):
        shard = indices[c * BPC:(c + 1) * BPC]
        idxw, order = _make_idxw(shard)
        in_maps.append({**common, "idxw": idxw})
        orders.append(order)

    res = run_bass_kernel_spmd(nc, in_maps, core_ids=list(range(N_CORES)),
                               trace=trace, **(trace_kwargs or {}))
    out = np.empty(B, np.float32)
    for c in range(N_CORES):
        out[c * BPC + orders[c]] = res.results[c]["out"]
    return out, res


def kernel(**inputs):
    out, _ = _run(inputs, trace=False)
    return out


# revision 6
# speedup vs baseline: 1.0362x; 1.0362x over previous
"""CoSTCo model kernel for 8x Trainium2 NeuronCores.

Math: out[b] = relu(wfc2 @ relu(wfc1 @ h2[b] + bfc1) + bfc2), where
  h2[b] = relu(Q02[i0[b]*64 + i2[b]] + Q1[i1[b]])
  Q_m   = relu(emb_m @ w1.T + b1) @ w2[:, :, m].T        (weight folding)
  Q02[i*64+j] = Q0[i] + Q2[j] + b2                       (pair fusion)

conv1 (over rank) and conv2 (over modes) act linearly on each gathered
embedding row, so they fold into per-table lookup matrices Q_m computed
once on the host (tables are tiny: 339/5825/64 rows). Modes 0 and 2 fuse
further into one 21696-row pair table, so the device does 2 dma_gathers
per batch element instead of 3.

Device per 512-batch block: 1 DVE add, 8 PE transposes into [channel,
batch] layout, relu(+bias) drains, and the MLP (256->256->1) on the
tensor engine.

Startup path: one packed idx DMA (p-major contiguous layout), two packed
const DMAs, so the first gather issues within a few microseconds. The
gather chunk is 2048 indices to amortize the ~1us fixed SWDGE
descriptor-generation cost per gather instruction.

Sharding: pure data parallel over the batch dim, 16384 elements per core.
"""

import sys
import types

sys.path.insert(0, "/opt/trn_rl_repo")

import ml_dtypes
import numpy as np

# ---------------------------------------------------------------- constants
B = 131072
N_CORES = 8
BPC = B // N_CORES          # 16384 batch elements per core
CHUNK = 1024                # idx per dma_gather instruction (Q7 scratch caps this)
RANK = 128
C = 256                     # channels
FIELD_DIMS = (339, 5825, 64)
F02 = FIELD_DIMS[0] * FIELD_DIMS[2]   # fused pair-table rows
NSWQ = 4                    # SWDGE queues in use
SCRATCH = 65536             # dynamic DMA scratch bytes per partition

TDT = "bf16"                # gather-table dtype
MDT = "bf16"                # matmul/activation dtype


def _install_ntff_hook():
    """antenv in this image lacks axon_hooks; inject it and register the
    ctypes NTFF profiling hook so trace=True works under axon."""
    import antenv

    if "antenv.axon_hooks" in sys.modules:
        return
    mod = types.ModuleType("antenv.axon_hooks")
    mod._hook = None
    mod.set_axon_ntff_profile_hook = lambda h: setattr(mod, "_hook", h)
    mod.get_axon_ntff_profile_hook = lambda: mod._hook
    sys.modules["antenv.axon_hooks"] = mod
    antenv.axon_hooks = mod
    try:
        from trn_agent_boot.trn_boot import _ntff_profile_via_ctypes

        mod._hook = _ntff_profile_via_ctypes("/opt/axon/libaxon_pjrt.so")
    except Exception:
        pass


_NC_CACHE = {}


def _build(bpc=BPC, chunk=CHUNK):
    """Build + compile the per-core Bass program. Identical on all cores;
    per-core data arrives via in_maps."""
    import concourse.tile as tile
    from concourse import bacc, mybir

    key = (bpc, chunk)
    if key in _NC_CACHE:
        return _NC_CACHE[key]

    f32 = mybir.dt.float32
    bf16 = mybir.dt.bfloat16
    i16 = mybir.dt.int16
    Alu = mybir.AluOpType
    Act = mybir.ActivationFunctionType
    nchunk = bpc // chunk
    nblk = chunk // 512
    ngrp = chunk // 128
    cw = chunk // 16            # idx columns per chunk (per table)
    iw = bpc // 16              # idx columns per table

    nc = bacc.Bacc("TRN2", target_bir_lowering=False, debug=False,
                   num_devices=N_CORES, num_swdge_queues=NSWQ,
                   dynamic_dma_scratch_size=SCRATCH)

    # DRAM inputs (per-core shards / replicated folded weights)
    q02_dram = nc.dram_tensor("q02", [F02, C], bf16, kind="ExternalInput")
    q1_dram = nc.dram_tensor("q1", [FIELD_DIMS[1], C], bf16,
                             kind="ExternalInput")
    # both tables' wrapped idx packed p-major: [128, 2*iw] (table-major cols)
    idx_dram = nc.dram_tensor("idxw", [128, 2 * iw], i16,
                              kind="ExternalInput")
    # packed consts: [128, 128 ident | 512 w1t | 2 w2t] bf16, [128, 3] f32
    cwb_dram = nc.dram_tensor("cwb", [128, 642], bf16, kind="ExternalInput")
    cbf_dram = nc.dram_tensor("cbf", [128, 3], f32, kind="ExternalInput")
    out_dram = nc.dram_tensor("out", [bpc], f32, kind="ExternalOutput")
    out_view = out_dram.ap().rearrange("(c n) -> c n", n=chunk)

    with tile.TileContext(nc) as tc:
        with (
            tc.tile_pool(name="const", bufs=1) as const_pool,
            tc.tile_pool(name="gat", bufs=4) as gat_pool,
            tc.tile_pool(name="sum", bufs=3) as sum_pool,
            tc.tile_pool(name="act", bufs=3) as act_pool,
            tc.tile_pool(name="stage", bufs=3) as stage_pool,
            tc.tile_pool(name="pt", bufs=4, space="PSUM") as pt_pool,
            tc.tile_pool(name="ph", bufs=3, space="PSUM") as ph_pool,
            tc.tile_pool(name="po", bufs=1, space="PSUM") as po_pool,
        ):
            # --- idx first (gates the first gather), then consts.
            # chunk-0 idx lives in its own small tiles so the first gathers
            # fire without waiting for the full idx load (tile-granular
            # dependency tracking would otherwise gate on the whole tile).
            idx_first, idx_rest = [], []
            for m in range(2):
                it = const_pool.tile([128, cw], i16, tag=f"idxf{m}")
                nc.sync.dma_start(it[:], idx_dram.ap()[:, m * iw:m * iw + cw])
                idx_first.append(it)
            for m in range(2):
                it = const_pool.tile([128, iw - cw], i16, tag=f"idxr{m}")
                nc.sync.dma_start(it[:],
                                  idx_dram.ap()[:, m * iw + cw:(m + 1) * iw])
                idx_rest.append(it)
            cwb = const_pool.tile([128, 642], bf16)
            nc.sync.dma_start(cwb[:], cwb_dram.ap())
            cbf = const_pool.tile([128, 3], f32)
            nc.sync.dma_start(cbf[:], cbf_dram.ap())

            ident = cwb[:, 0:128]
            w1t = [cwb[:, 128 + j * C:128 + (j + 1) * C] for j in range(2)]
            w2t = cwb[:, 640:642]
            b1s = cbf[:, 0:2]
            b3s = cbf[0:1, 2:3]

            for ch in range(nchunk):
                # --- gather table rows for this chunk: [128, ngrp, 256]
                g = []
                for m, src in enumerate((q02_dram, q1_dram)):
                    dst = gat_pool.tile([128, ngrp, C], bf16, tag=f"g{m}")
                    src_idx = (idx_first[m][:] if ch == 0 else
                               idx_rest[m][:, (ch - 1) * cw:ch * cw])
                    nc.gpsimd.dma_gather(
                        dst[:], src.ap(),
                        src_idx,
                        chunk, chunk, C,
                        queue_num=(2 * ch + m) % NSWQ,
                    )
                    g.append(dst)

                stage = stage_pool.tile([1, chunk], f32)
                for blk in range(nblk):
                    gs = slice(4 * blk, 4 * blk + 4)
                    # --- s = g02 + g1  (row layout [batch, channel])
                    s = sum_pool.tile([128, 4, C], bf16)
                    nc.vector.tensor_tensor(s[:], g[0][:, gs, :],
                                            g[1][:, gs, :], Alu.add)
                    # --- transpose to [channel, batch], 2 halves of 128
                    h2 = []
                    for h in range(2):
                        ps = pt_pool.tile([128, 512], bf16, tag="pt")
                        for grp in range(4):
                            nc.tensor.transpose(
                                ps[:, grp * 128:(grp + 1) * 128],
                                s[:, grp, h * 128:(h + 1) * 128],
                                ident,
                            )
                        # --- h2 = relu(sum)  (b2 folded into q02 on host)
                        hs = act_pool.tile([128, 512], bf16, tag=f"h2{h}")
                        nc.scalar.activation(hs[:], ps[:], Act.Relu)
                        h2.append(hs[:])
                    # --- fc1: h3 = relu(wfc1 @ h2 + bfc1), 2 output halves
                    h3 = []
                    for h in range(2):
                        ph = ph_pool.tile([128, 512], f32, tag="ph")
                        for j in range(2):
                            nc.tensor.matmul(
                                ph[:],
                                w1t[j][:, h * 128:(h + 1) * 128],
                                h2[j],
                                start=(j == 0), stop=(j == 1),
                            )
                        hs = act_pool.tile([128, 512], bf16, tag=f"h3{h}")
                        nc.vector.tensor_scalar(hs[:], ph[:],
                                                b1s[:, h:h + 1], 0.0,
                                                Alu.add, Alu.max)
                        h3.append(hs)
                    # --- fc2: out = relu(wfc2 @ h3 + bfc2)
                    po = po_pool.tile([128, 512], f32, tag="po")
                    for j in range(2):
                        nc.tensor.matmul(po[0:1, :],
                                         w2t[:, j:j + 1],
                                         h3[j][:],
                                         start=(j == 0), stop=(j == 1))
                    nc.scalar.activation(
                        stage[0:1, blk * 512:(blk + 1) * 512], po[0:1, :],
                        Act.Relu, bias=b3s)
                nc.sync.dma_start(out_view[ch:ch + 1, :], stage[:])

    nc.compile()
    _NC_CACHE[key] = nc
    return nc


def _fold_tables(inputs):
    """Q_m = relu(emb_m @ w1.T + b1) @ w2[:,:,m].T in float64, then the
    mode-0/2 pair fusion Q02[i*64+j] = Q0[i] + Q2[j] + b2."""
    w1_ = np.asarray(inputs["w1"]).astype(np.float64)
    b1_ = np.asarray(inputs["b1"]).astype(np.float64)
    w2 = np.asarray(inputs["w2"])
    qs = []
    for m, emb in enumerate((inputs["emb0"], inputs["emb1"], inputs["emb2"])):
        r = np.maximum(np.asarray(emb).astype(np.float64) @ w1_.T + b1_, 0.0)
        qs.append(r @ w2[:, :, m].astype(np.float64).T)
    q02 = (qs[0][:, None, :] + qs[2][None, :, :]
           + np.asarray(inputs["b2"]).astype(np.float64)).reshape(F02, C)
    return q02, qs[1]


def _make_common(inputs):
    bf = ml_dtypes.bfloat16
    q02, q1 = _fold_tables(inputs)
    ident = np.eye(128, dtype=bf)
    w1t = np.asarray(inputs["wfc1"]).T.astype(bf).reshape(2, 128, C)
    # cwb free layout: [ident 128 | w1t[j=0] 256 | w1t[j=1] 256 | w2t 2]
    cwb = np.concatenate(
        [ident, w1t[0], w1t[1],
         np.asarray(inputs["wfc2"]).reshape(C).astype(bf).reshape(2, 128).T],
        axis=1)
    cbf = np.zeros((128, 3), np.float32)
    cbf[:, 0:2] = np.asarray(inputs["bfc1"]).astype(np.float32).reshape(2, 128).T
    cbf[0, 2] = float(np.asarray(inputs["bfc2"]).reshape(()))
    return {
        "q02": np.ascontiguousarray(q02.astype(bf)),
        "q1": np.ascontiguousarray(q1.astype(bf)),
        "cwb": np.ascontiguousarray(cwb),
        "cbf": cbf,
    }


def _wrap_idx(idx, chunk):
    """Wrap a 1-D int array into dma_gather's [128, n/16] int16 layout,
    chunk by chunk: logical position k of chunk c lives at
    [k % 16, c*chunk/16 + k // 16], replicated across the 8 Q7 cores."""
    n = idx.shape[0]
    w = (idx.reshape(n // chunk, chunk // 16, 16)
         .transpose(0, 2, 1).reshape(n // chunk, 16, chunk // 16))
    wrapped = np.concatenate(list(w), axis=1).astype(np.int16)  # [16, n/16]
    return np.tile(wrapped, (8, 1))                             # [128, n/16]


def _make_idxw(shard, chunk=CHUNK):
    """shard: [n, 3] int indices -> ([128, 2*n/16] int16 packed layout,
    order): cols [0, n/16) are the fused mode-0/2 index, cols [n/16, 2n/16)
    the mode-1 index. The batch is sorted by the fused index so the
    big-table HBM reads are sequential-ish; `order` maps device position ->
    original row (undo with out[order] = device_out)."""
    i02 = np.asarray(shard[:, 0]).astype(np.int64) * FIELD_DIMS[2] \
        + np.asarray(shard[:, 2])
    i1 = np.asarray(shard[:, 1]).astype(np.int64)
    order = np.arange(i02.shape[0])
    return np.concatenate([_wrap_idx(i02[order], chunk),
                           _wrap_idx(i1[order], chunk)], axis=1), order


def _run(inputs, trace=False, trace_kwargs=None):
    _install_ntff_hook()
    from concourse.bass_utils import run_bass_kernel_spmd

    nc = _build()
    common = _make_common(inputs)
    indices = np.asarray(inputs["indices"])
    in_maps, orders = [], []
    for c in range(N_CORES):
        shard = indices[c * BPC:(c + 1) * BPC]
        idxw, order = _make_idxw(shard)
        in_maps.append({**common, "idxw": idxw})
        orders.append(order)

    res = run_bass_kernel_spmd(nc, in_maps, core_ids=list(range(N_CORES)),
                               trace=trace, **(trace_kwargs or {}))
    out = np.empty(B, np.float32)
    for c in range(N_CORES):
        out[c * BPC + orders[c]] = res.results[c]["out"]
    return out, res


def kernel(**inputs):
    out, _ = _run(inputs, trace=False)
    return out


# revision 7
# speedup vs baseline: 1.0638x; 1.0266x over previous
"""CoSTCo model kernel for 8x Trainium2 NeuronCores.

Math: out[b] = relu(wfc2 @ relu(wfc1 @ h2[b] + bfc1) + bfc2), where
  h2[b] = relu(Q02[i0[b]*64 + i2[b]] + Q1[i1[b]])
  Q_m   = relu(emb_m @ w1.T + b1) @ w2[:, :, m].T        (weight folding)
  Q02[i*64+j] = Q0[i] + Q2[j] + b2                       (pair fusion)

conv1 (over rank) and conv2 (over modes) act linearly on each gathered
embedding row, so they fold into per-table lookup matrices Q_m computed
once on the host (tables are tiny: 339/5825/64 rows). Modes 0 and 2 fuse
further into one 21696-row pair table, so the device does 2 dma_gathers
per batch element instead of 3.

Device per 512-batch block: 1 DVE add, 8 PE transposes into [channel,
batch] layout, relu(+bias) drains, and the MLP (256->256->1) on the
tensor engine.

Startup path: one packed idx DMA (p-major contiguous layout), two packed
const DMAs, so the first gather issues within a few microseconds. The
gather chunk is 2048 indices to amortize the ~1us fixed SWDGE
descriptor-generation cost per gather instruction.

Sharding: pure data parallel over the batch dim, 16384 elements per core.
"""

import sys
import types

sys.path.insert(0, "/opt/trn_rl_repo")

import ml_dtypes
import numpy as np

# ---------------------------------------------------------------- constants
B = 131072
N_CORES = 8
BPC = B // N_CORES          # 16384 batch elements per core
CHUNK = 1024                # idx per dma_gather instruction (Q7 scratch caps this)
RANK = 128
C = 256                     # channels
FIELD_DIMS = (339, 5825, 64)
F02 = FIELD_DIMS[0] * FIELD_DIMS[2]   # fused pair-table rows
NSWQ = 4                    # SWDGE queues in use
SCRATCH = 65536             # dynamic DMA scratch bytes per partition

TDT = "bf16"                # gather-table dtype
MDT = "bf16"                # matmul/activation dtype


def _install_ntff_hook():
    """antenv in this image lacks axon_hooks; inject it and register the
    ctypes NTFF profiling hook so trace=True works under axon."""
    import antenv

    if "antenv.axon_hooks" in sys.modules:
        return
    mod = types.ModuleType("antenv.axon_hooks")
    mod._hook = None
    mod.set_axon_ntff_profile_hook = lambda h: setattr(mod, "_hook", h)
    mod.get_axon_ntff_profile_hook = lambda: mod._hook
    sys.modules["antenv.axon_hooks"] = mod
    antenv.axon_hooks = mod
    try:
        from trn_agent_boot.trn_boot import _ntff_profile_via_ctypes

        mod._hook = _ntff_profile_via_ctypes("/opt/axon/libaxon_pjrt.so")
    except Exception:
        pass


_NC_CACHE = {}


def _build(bpc=BPC, chunk=CHUNK):
    """Build + compile the per-core Bass program. Identical on all cores;
    per-core data arrives via in_maps."""
    import concourse.tile as tile
    from concourse import bacc, mybir

    key = (bpc, chunk)
    if key in _NC_CACHE:
        return _NC_CACHE[key]

    f32 = mybir.dt.float32
    bf16 = mybir.dt.bfloat16
    i16 = mybir.dt.int16
    Alu = mybir.AluOpType
    Act = mybir.ActivationFunctionType
    nchunk = bpc // chunk
    nblk = chunk // 512
    ngrp = chunk // 128
    cw = chunk // 16            # idx columns per chunk (per table)
    iw = bpc // 16              # idx columns per table

    nc = bacc.Bacc("TRN2", target_bir_lowering=False, debug=False,
                   num_devices=N_CORES, num_swdge_queues=NSWQ,
                   dynamic_dma_scratch_size=SCRATCH)

    # DRAM inputs (per-core shards / replicated folded weights)
    q02_dram = nc.dram_tensor("q02", [F02, C], bf16, kind="ExternalInput")
    q1_dram = nc.dram_tensor("q1", [FIELD_DIMS[1], C], bf16,
                             kind="ExternalInput")
    # both tables' wrapped idx packed p-major: [128, 2*iw] (table-major cols)
    idx_dram = nc.dram_tensor("idxw", [128, 2 * iw], i16,
                              kind="ExternalInput")
    # packed consts: [128, 128 ident | 512 w1t | 2 w2t] bf16, [128, 3] f32
    cwb_dram = nc.dram_tensor("cwb", [128, 642], bf16, kind="ExternalInput")
    cbf_dram = nc.dram_tensor("cbf", [128, 3], f32, kind="ExternalInput")
    out_dram = nc.dram_tensor("out", [bpc], f32, kind="ExternalOutput")
    out_view = out_dram.ap().rearrange("(c n) -> c n", n=chunk)

    with tile.TileContext(nc) as tc:
        with (
            tc.tile_pool(name="const", bufs=1) as const_pool,
            tc.tile_pool(name="gat", bufs=4) as gat_pool,
            tc.tile_pool(name="sum", bufs=3) as sum_pool,
            tc.tile_pool(name="act", bufs=3) as act_pool,
            tc.tile_pool(name="stage", bufs=3) as stage_pool,
            tc.tile_pool(name="pt", bufs=4, space="PSUM") as pt_pool,
            tc.tile_pool(name="ph", bufs=3, space="PSUM") as ph_pool,
            tc.tile_pool(name="po", bufs=1, space="PSUM") as po_pool,
        ):
            # --- idx first (gates the first gather), then consts.
            # chunk-0 idx lives in its own small tiles so the first gathers
            # fire without waiting for the full idx load (tile-granular
            # dependency tracking would otherwise gate on the whole tile).
            idx_first, idx_rest = [], []
            for m in range(2):
                it = const_pool.tile([128, cw], i16, tag=f"idxf{m}")
                nc.sync.dma_start(it[:], idx_dram.ap()[:, m * iw:m * iw + cw])
                idx_first.append(it)
            for m in range(2):
                it = const_pool.tile([128, iw - cw], i16, tag=f"idxr{m}")
                nc.sync.dma_start(it[:],
                                  idx_dram.ap()[:, m * iw + cw:(m + 1) * iw])
                idx_rest.append(it)
            cwb = const_pool.tile([128, 642], bf16)
            nc.sync.dma_start(cwb[:], cwb_dram.ap())
            cbf = const_pool.tile([128, 3], f32)
            nc.sync.dma_start(cbf[:], cbf_dram.ap())

            # one shared register for every gather's num_idxs; per-gather
            # to_reg MOVEs inherit the gather's dependencies and block the
            # in-order GPSIMD queue ahead of the first gather.
            nidx_reg = nc.gpsimd.to_reg(chunk)

            ident = cwb[:, 0:128]
            w1t = [cwb[:, 128 + j * C:128 + (j + 1) * C] for j in range(2)]
            w2t = cwb[:, 640:642]
            b1s = cbf[:, 0:2]
            b3s = cbf[0:1, 2:3]

            for ch in range(nchunk):
                # --- gather table rows for this chunk: [128, ngrp, 256]
                g = []
                for m, src in enumerate((q02_dram, q1_dram)):
                    dst = gat_pool.tile([128, ngrp, C], bf16, tag=f"g{m}")
                    src_idx = (idx_first[m][:] if ch == 0 else
                               idx_rest[m][:, (ch - 1) * cw:ch * cw])
                    nc.gpsimd.dma_gather(
                        dst[:], src.ap(),
                        src_idx,
                        chunk, nidx_reg, C,
                        queue_num=(2 * ch + m) % NSWQ,
                    )
                    g.append(dst)

                stage = stage_pool.tile([1, chunk], f32)
                for blk in range(nblk):
                    gs = slice(4 * blk, 4 * blk + 4)
                    # --- s = g02 + g1  (row layout [batch, channel])
                    s = sum_pool.tile([128, 4, C], bf16)
                    nc.vector.tensor_tensor(s[:], g[0][:, gs, :],
                                            g[1][:, gs, :], Alu.add)
                    # --- transpose to [channel, batch], 2 halves of 128
                    h2 = []
                    for h in range(2):
                        ps = pt_pool.tile([128, 512], bf16, tag="pt")
                        for grp in range(4):
                            nc.tensor.transpose(
                                ps[:, grp * 128:(grp + 1) * 128],
                                s[:, grp, h * 128:(h + 1) * 128],
                                ident,
                            )
                        # --- h2 = relu(sum)  (b2 folded into q02 on host)
                        hs = act_pool.tile([128, 512], bf16, tag=f"h2{h}")
                        nc.scalar.activation(hs[:], ps[:], Act.Relu)
                        h2.append(hs[:])
                    # --- fc1: h3 = relu(wfc1 @ h2 + bfc1), 2 output halves
                    h3 = []
                    for h in range(2):
                        ph = ph_pool.tile([128, 512], f32, tag="ph")
                        for j in range(2):
                            nc.tensor.matmul(
                                ph[:],
                                w1t[j][:, h * 128:(h + 1) * 128],
                                h2[j],
                                start=(j == 0), stop=(j == 1),
                            )
                        hs = act_pool.tile([128, 512], bf16, tag=f"h3{h}")
                        nc.vector.tensor_scalar(hs[:], ph[:],
                                                b1s[:, h:h + 1], 0.0,
                                                Alu.add, Alu.max)
                        h3.append(hs)
                    # --- fc2: out = relu(wfc2 @ h3 + bfc2)
                    po = po_pool.tile([128, 512], f32, tag="po")
                    for j in range(2):
                        nc.tensor.matmul(po[0:1, :],
                                         w2t[:, j:j + 1],
                                         h3[j][:],
                                         start=(j == 0), stop=(j == 1))
                    nc.scalar.activation(
                        stage[0:1, blk * 512:(blk + 1) * 512], po[0:1, :],
                        Act.Relu, bias=b3s)
                nc.sync.dma_start(out_view[ch:ch + 1, :], stage[:])

    nc.compile()
    _NC_CACHE[key] = nc
    return nc


def _fold_tables(inputs):
    """Q_m = relu(emb_m @ w1.T + b1) @ w2[:,:,m].T in float64, then the
    mode-0/2 pair fusion Q02[i*64+j] = Q0[i] + Q2[j] + b2."""
    w1_ = np.asarray(inputs["w1"]).astype(np.float64)
    b1_ = np.asarray(inputs["b1"]).astype(np.float64)
    w2 = np.asarray(inputs["w2"])
    qs = []
    for m, emb in enumerate((inputs["emb0"], inputs["emb1"], inputs["emb2"])):
        r = np.maximum(np.asarray(emb).astype(np.float64) @ w1_.T + b1_, 0.0)
        qs.append(r @ w2[:, :, m].astype(np.float64).T)
    q02 = (qs[0][:, None, :] + qs[2][None, :, :]
           + np.asarray(inputs["b2"]).astype(np.float64)).reshape(F02, C)
    return q02, qs[1]


def _make_common(inputs):
    bf = ml_dtypes.bfloat16
    q02, q1 = _fold_tables(inputs)
    ident = np.eye(128, dtype=bf)
    w1t = np.asarray(inputs["wfc1"]).T.astype(bf).reshape(2, 128, C)
    # cwb free layout: [ident 128 | w1t[j=0] 256 | w1t[j=1] 256 | w2t 2]
    cwb = np.concatenate(
        [ident, w1t[0], w1t[1],
         np.asarray(inputs["wfc2"]).reshape(C).astype(bf).reshape(2, 128).T],
        axis=1)
    cbf = np.zeros((128, 3), np.float32)
    cbf[:, 0:2] = np.asarray(inputs["bfc1"]).astype(np.float32).reshape(2, 128).T
    cbf[0, 2] = float(np.asarray(inputs["bfc2"]).reshape(()))
    return {
        "q02": np.ascontiguousarray(q02.astype(bf)),
        "q1": np.ascontiguousarray(q1.astype(bf)),
        "cwb": np.ascontiguousarray(cwb),
        "cbf": cbf,
    }


def _wrap_idx(idx, chunk):
    """Wrap a 1-D int array into dma_gather's [128, n/16] int16 layout,
    chunk by chunk: logical position k of chunk c lives at
    [k % 16, c*chunk/16 + k // 16], replicated across the 8 Q7 cores."""
    n = idx.shape[0]
    w = (idx.reshape(n // chunk, chunk // 16, 16)
         .transpose(0, 2, 1).reshape(n // chunk, 16, chunk // 16))
    wrapped = np.concatenate(list(w), axis=1).astype(np.int16)  # [16, n/16]
    return np.tile(wrapped, (8, 1))                             # [128, n/16]


def _make_idxw(shard, chunk=CHUNK):
    """shard: [n, 3] int indices -> ([128, 2*n/16] int16 packed layout,
    order): cols [0, n/16) are the fused mode-0/2 index, cols [n/16, 2n/16)
    the mode-1 index. The batch is sorted by the fused index so the
    big-table HBM reads are sequential-ish; `order` maps device position ->
    original row (undo with out[order] = device_out)."""
    i02 = np.asarray(shard[:, 0]).astype(np.int64) * FIELD_DIMS[2] \
        + np.asarray(shard[:, 2])
    i1 = np.asarray(shard[:, 1]).astype(np.int64)
    order = np.arange(i02.shape[0])
    return np.concatenate([_wrap_idx(i02[order], chunk),
                           _wrap_idx(i1[order], chunk)], axis=1), order


def _run(inputs, trace=False, trace_kwargs=None):
    _install_ntff_hook()
    from concourse.bass_utils import run_bass_kernel_spmd

    nc = _build()
    common = _make_common(inputs)
    indices = np.asarray(inputs["indices"])
    in_maps, orders = [], []
    for c in range(N_CORES):
        shard = indices[c * BPC:(c + 1) * BPC]
        idxw, order = _make_idxw(shard)
        in_maps.append({**common, "idxw": idxw})
        orders.append(order)

    res = run_bass_kernel_spmd(nc, in_maps, core_ids=list(range(N_CORES)),
                               trace=trace, **(trace_kwargs or {}))
    out = np.empty(B, np.float32)
    for c in range(N_CORES):
        out[c * BPC + orders[c]] = res.results[c]["out"]
    return out, res


def kernel(**inputs):
    out, _ = _run(inputs, trace=False)
    return out
